# revision 4
# baseline (speedup 1.0000x reference)
"""Trainium2 Bass kernel for nn_CorrelationHead.

Computes: SpatialCorrelationSampler(patch_size=21, dilation_patch=2) over
[B,C,7,7] patch pairs, flattened through a 21609->4 Linear head.

Reformulation (validated to ~1e-6 rel err vs the jax reference):
  corr[b,p,q,i,j] = sum_c patch1[b,c,i,j] * patch2[b,c, i+2p-20, j+2q-20]
  out[b,n]  = sum_{pqij} w[n,pqij] * corr[b,pqij] + bias[n]
            = sum_{ij,yx} A[b][ij,yx] * W3[n,ij,yx] + bias[n]
  where A[b] = P1[b]^T @ P2[b]  (Gram over c=128; ij,yx in [0,49))
  and   W3[n,ij,yx] = w[n, (p,q,i,j)] gathered with y=2p+i-20, x=2q+j-20
  (displacements landing outside the 7x7 patch multiply zero padding and
  are dropped: W3 holds only the 625 surviving taps per (n)).

Per-core mapping (pure data parallelism over B, 64 samples/core):
  stage 1: 64 matmuls  lhsT=P2[b] [c=128, yx=49], rhs=P1[b] [c=128, ij=49]
           -> PSUM [yx=49, ij=49] = A^T[b]; copied into SBUF
           acat[yx, b, ij].
  stage 2: 49 accumulating matmuls over ij: lhsT=Wst2[:, ij*4:(ij+1)*4]
           ([yx=49, n=4]), rhs=acat[:, :, ij] ([yx=49, b=64]) -> PSUM
           [4, 64]; bias added on the DVE copy out.
"""

import os

import numpy as np

import concourse.bass as bass
import concourse.mybir as mybir
import concourse.tile as tile
from concourse import bacc
from concourse.bass_utils import run_bass_kernel_spmd

N_CORES = 8
B, C, HW = 512, 128, 49
BS = B // N_CORES  # 64 samples per core
PAD = 20
GROUPS = 4         # input DMA batching: GROUPS loads of GB samples each
GB = BS // GROUPS

_F32 = mybir.dt.float32


def _build_wst2(w_bbox: np.ndarray) -> np.ndarray:
    """Gather w_bbox [4, 21609] into Wst2 [yx=49, ij*4+n] (fp32)."""
    W3 = np.zeros((4, 49, 49), np.float32)
    for i in range(7):
        for j in range(7):
            for y in range(7):
                for x in range(7):
                    if (y - i) % 2 == 0 and (x - j) % 2 == 0:
                        p = (y - i + PAD) // 2
                        q = (x - j + PAD) // 2
                        W3[:, i * 7 + j, y * 7 + x] = w_bbox[
                            :, ((p * 21 + q) * 7 + i) * 7 + j
                        ]
    # Wst2[yx, ij*4 + n] = W3[n, ij, yx]
    return np.ascontiguousarray(W3.transpose(2, 1, 0).reshape(49, 196))


def build_nc() -> bass.Bass:
    nc = bacc.Bacc("TRN2", target_bir_lowering=False, debug=False)
    p1 = nc.dram_tensor("p1", [BS, C, HW], _F32, kind="ExternalInput")
    p2 = nc.dram_tensor("p2", [BS, C, HW], _F32, kind="ExternalInput")
    wst2 = nc.dram_tensor("wst2", [49, 196], _F32, kind="ExternalInput")
    bias = nc.dram_tensor("bias", [4, 1], _F32, kind="ExternalInput")
    out = nc.dram_tensor("out", [4, BS], _F32, kind="ExternalOutput")

    with tile.TileContext(nc) as tc:
        with (
            tc.tile_pool(name="consts", bufs=1) as consts,
            tc.tile_pool(name="loads", bufs=3) as loads,
            tc.tile_pool(name="acat", bufs=1) as acat_pool,
            tc.tile_pool(name="ps1", bufs=6, space=bass.MemorySpace.PSUM) as ps1,
            tc.tile_pool(name="ps2", bufs=1, space=bass.MemorySpace.PSUM) as ps2,
        ):
            w_t = consts.tile([49, 196], _F32)
            nc.sync.dma_start(out=w_t[:], in_=wst2[:])
            b_t = consts.tile([4, 1], _F32)
            nc.sync.dma_start(out=b_t[:], in_=bias[:])

            acat = acat_pool.tile([49, BS, HW], _F32)  # [yx, b, ij]

            p1r = p1[:].rearrange("b c i -> c b i")
            p2r = p2[:].rearrange("b c i -> c b i")
            for g in range(GROUPS):
                t1 = loads.tile([C, GB, HW], _F32, tag="t1")
                t2 = loads.tile([C, GB, HW], _F32, tag="t2")
                nc.sync.dma_start(out=t1[:], in_=p1r[:, g * GB : (g + 1) * GB, :])
                nc.sync.dma_start(out=t2[:], in_=p2r[:, g * GB : (g + 1) * GB, :])
                for k in range(GB):
                    bb = g * GB + k
                    ps = ps1.tile([49, HW], _F32)
                    nc.tensor.matmul(
                        ps[:], t2[:, k, :], t1[:, k, :], start=True, stop=True
                    )
                    nc.vector.tensor_copy(acat[:, bb, :], ps[:])

            out_ps = ps2.tile([4, BS], _F32)
            for ij in range(HW):
                nc.tensor.matmul(
                    out_ps[:],
                    w_t[:, ij * 4 : (ij + 1) * 4],
                    acat[:, :, ij],
                    start=(ij == 0),
                    stop=(ij == HW - 1),
                )
            out_sb = consts.tile([4, BS], _F32)
            nc.vector.tensor_scalar_add(out_sb[:], out_ps[:], b_t[:])
            nc.sync.dma_start(out=out[:], in_=out_sb[:])
    nc.compile()
    return nc


def _prep_inputs(inputs):
    p1 = np.ascontiguousarray(np.asarray(inputs["patch1"], np.float32)).reshape(
        B, C, HW
    )
    p2 = np.ascontiguousarray(np.asarray(inputs["patch2"], np.float32)).reshape(
        B, C, HW
    )
    wst2 = _build_wst2(np.asarray(inputs["w_bbox"], np.float32))
    bias = np.ascontiguousarray(
        np.asarray(inputs["b_bbox"], np.float32).reshape(4, 1)
    )
    in_maps = []
    for c in range(N_CORES):
        sl = slice(c * BS, (c + 1) * BS)
        in_maps.append(
            {
                "p1": np.ascontiguousarray(p1[sl]),
                "p2": np.ascontiguousarray(p2[sl]),
                "wst2": wst2,
                "bias": bias,
            }
        )
    return in_maps


def _run(inputs, trace: bool = False):
    nc = build_nc()
    in_maps = _prep_inputs(inputs)
    res = run_bass_kernel_spmd(
        nc, in_maps, core_ids=list(range(N_CORES)), trace=trace
    )
    out = np.concatenate(
        [res.results[c]["out"].T for c in range(N_CORES)], axis=0
    ).astype(np.float32)
    return out, res


def kernel(**inputs) -> np.ndarray:
    out, _ = _run(inputs, trace=False)
    return out


# revision 5
# speedup vs baseline: 1.2402x; 1.2402x over previous
"""Trainium2 Bass kernel for nn_CorrelationHead.

Computes: SpatialCorrelationSampler(patch_size=21, dilation_patch=2) over
[B,C,7,7] patch pairs, flattened through a 21609->4 Linear head.

Reformulation (validated to ~1e-6 rel err vs the jax reference in fp32):
  corr[b,p,q,i,j] = sum_c patch1[b,c,i,j] * patch2[b,c, i+2p-20, j+2q-20]
  out[b,n]  = sum_{pqij} w[n,pqij] * corr[b,pqij] + bias[n]
            = sum_{ij,yx} A[b][ij,yx] * W3[n,ij,yx] + bias[n]
  where A[b] = P1[b]^T @ P2[b]  (Gram over c=128; ij,yx in [0,49))
  and   W3[n,ij,yx] = w[n,(p,q,i,j)] gathered with y=2p+i-20, x=2q+j-20
  (displacements landing outside the 7x7 patch multiply zero padding and
  drop out; W3 keeps the 625 surviving taps).

Per-core mapping (pure data parallelism over B, 64 samples/core), bf16:
  host: pack both patches bf16-cast and channel-pair-interleaved into
        Y[b, p, 196] with Y[b,p] = [p1[b,2p] | p2[b,2p] | p1[b,2p+1] |
        p2[b,2p+1]] -> 392B contiguous DMA descriptors per (b,p).
  stage 1: per b, two accumulating matmuls (c-even, c-odd; K=64):
        lhsT=p2-slice [64, yx=49], rhs=p1-slice [64, ij=49]
        -> PSUM [yx=49, ij=49] = A^T[b]; 4 samples per PSUM tile,
        batch-copied (f32->bf16) into SBUF acat[yx, b, ij].
  stage 2: 49 accumulating matmuls over ij: lhsT=Wst2[:, ij*4:(ij+1)*4]
        ([yx=49, n=4] bf16), rhs=acat[:, :, ij] ([yx=49, b=64] bf16)
        -> PSUM [4, 64] f32; bias added on the DVE copy out.
"""

import numpy as np

import concourse.bass as bass
import concourse.mybir as mybir
import concourse.tile as tile
from concourse import bacc
from concourse.bass_utils import run_bass_kernel_spmd

N_CORES = 8
B, C, HW = 512, 128, 49
BS = B // N_CORES   # 64 samples per core
CP = C // 2         # 64 partitions, 2 channels each
FW = 4 * HW         # 196 packed row: p1/even, p2/even, p1/odd, p2/odd
PAD = 20
GROUPS = 4          # input DMA batching: GROUPS loads of GB samples each
GB = BS // GROUPS
PSB = 4             # samples per PSUM tile / per batched copy

_F32 = mybir.dt.float32
_BF16 = mybir.dt.bfloat16


def _build_wst2(w_bbox: np.ndarray) -> np.ndarray:
    """Gather w_bbox [4, 21609] into Wst2 [yx=49, ij*4+n] (fp32)."""
    W3 = np.zeros((4, 49, 49), np.float32)
    for i in range(7):
        for j in range(7):
            for y in range(7):
                for x in range(7):
                    if (y - i) % 2 == 0 and (x - j) % 2 == 0:
                        p = (y - i + PAD) // 2
                        q = (x - j + PAD) // 2
                        W3[:, i * 7 + j, y * 7 + x] = w_bbox[
                            :, ((p * 21 + q) * 7 + i) * 7 + j
                        ]
    # Wst2[yx, ij*4 + n] = W3[n, ij, yx]
    return np.ascontiguousarray(W3.transpose(2, 1, 0).reshape(49, 196))


def build_nc() -> bass.Bass:
    nc = bacc.Bacc("TRN2", target_bir_lowering=False, debug=False)
    pp = nc.dram_tensor("pp", [BS, CP, FW], _BF16, kind="ExternalInput")
    wst2 = nc.dram_tensor("wst2", [49, 196], _BF16, kind="ExternalInput")
    bias = nc.dram_tensor("bias", [4, 1], _F32, kind="ExternalInput")
    out = nc.dram_tensor("out", [4, BS], _F32, kind="ExternalOutput")

    with tile.TileContext(nc) as tc:
        with (
            tc.tile_pool(name="consts", bufs=1) as consts,
            tc.tile_pool(name="loads", bufs=2) as loads,
            tc.tile_pool(name="acat", bufs=1) as acat_pool,
        ):
            w_t = consts.tile([49, 196], _BF16)
            nc.sync.dma_start(out=w_t[:], in_=wst2[:])
            b_t = consts.tile([4, 1], _F32)
            nc.sync.dma_start(out=b_t[:], in_=bias[:])

            acat = acat_pool.tile([49, BS, HW], _BF16)  # [yx, b, ij]

            ppr = pp[:].rearrange("b p f -> p b f")
            with tc.tile_pool(
                name="ps1", bufs=2, space=bass.MemorySpace.PSUM
            ) as ps1:
                ps = None
                for g in range(GROUPS):
                    t = loads.tile([CP, GB, FW], _BF16, tag="t")
                    eng = nc.sync if g % 2 == 0 else nc.scalar
                    eng.dma_start(out=t[:], in_=ppr[:, g * GB : (g + 1) * GB, :])
                    for k in range(GB):
                        bb = g * GB + k
                        j = bb % PSB
                        if j == 0:
                            ps = ps1.tile([49, PSB, 512], _F32)
                        nc.tensor.matmul(
                            ps[:, j, 0:HW],
                            t[:, k, 49:98],     # p2, even channels
                            t[:, k, 0:49],      # p1, even channels
                            start=True,
                            stop=False,
                        )
                        nc.tensor.matmul(
                            ps[:, j, 0:HW],
                            t[:, k, 147:196],   # p2, odd channels
                            t[:, k, 98:147],    # p1, odd channels
                            start=False,
                            stop=True,
                        )
                        if j == PSB - 1:
                            nc.vector.tensor_copy(
                                acat[:, bb - (PSB - 1) : bb + 1, :],
                                ps[:, :, 0:HW],
                            )
            with tc.tile_pool(
                name="ps2", bufs=1, space=bass.MemorySpace.PSUM
            ) as ps2:
                out_ps = ps2.tile([4, BS], _F32)
                for ij in range(HW):
                    nc.tensor.matmul(
                        out_ps[:],
                        w_t[:, ij * 4 : (ij + 1) * 4],
                        acat[:, :, ij],
                        start=(ij == 0),
                        stop=(ij == HW - 1),
                    )
                out_sb = consts.tile([4, BS], _F32)
                nc.vector.tensor_scalar_add(out_sb[:], out_ps[:], b_t[:])
                nc.sync.dma_start(out=out[:], in_=out_sb[:])
    nc.compile()
    return nc


def _prep_inputs(inputs):
    import ml_dtypes

    p1 = np.asarray(inputs["patch1"], np.float32).reshape(B, C, HW)
    p2 = np.asarray(inputs["patch2"], np.float32).reshape(B, C, HW)
    bf = ml_dtypes.bfloat16
    Y = np.empty((B, CP, FW), bf)
    Y[:, :, 0:49] = p1[:, 0::2, :]
    Y[:, :, 49:98] = p2[:, 0::2, :]
    Y[:, :, 98:147] = p1[:, 1::2, :]
    Y[:, :, 147:196] = p2[:, 1::2, :]
    wst2 = _build_wst2(np.asarray(inputs["w_bbox"], np.float32)).astype(bf)
    bias = np.ascontiguousarray(
        np.asarray(inputs["b_bbox"], np.float32).reshape(4, 1)
    )
    in_maps = []
    for c in range(N_CORES):
        sl = slice(c * BS, (c + 1) * BS)
        in_maps.append(
            {
                "pp": np.ascontiguousarray(Y[sl]),
                "wst2": wst2,
                "bias": bias,
            }
        )
    return in_maps


def _run(inputs, trace: bool = False):
    nc = build_nc()
    in_maps = _prep_inputs(inputs)
    res = run_bass_kernel_spmd(
        nc, in_maps, core_ids=list(range(N_CORES)), trace=trace
    )
    out = np.concatenate(
        [res.results[c]["out"].T for c in range(N_CORES)], axis=0
    ).astype(np.float32)
    return out, res


def kernel(**inputs) -> np.ndarray:
    out, _ = _run(inputs, trace=False)
    return out


# revision 9
# speedup vs baseline: 1.5888x; 1.2810x over previous
"""Trainium2 Bass kernel for nn_CorrelationHead.

Computes: SpatialCorrelationSampler(patch_size=21, dilation_patch=2) over
[B,C,7,7] patch pairs, flattened through a 21609->4 Linear head.

Reformulation (validated to ~1e-6 rel err vs the jax reference in fp32):
  corr[b,p,q,i,j] = sum_c patch1[b,c,i,j] * patch2[b,c, i+2p-20, j+2q-20]
  out[b,n]  = sum_{pqij} w[n,pqij] * corr[b,pqij] + bias[n]
            = sum_{ij,yx} A[b][ij,yx] * W3[n,ij,yx] + bias[n]
  where A[b] = P1[b]^T @ P2[b]  (Gram over c=128; ij,yx in [0,49))
  and   W3[n,ij,yx] = w[n,(p,q,i,j)] gathered with y=2p+i-20, x=2q+j-20
  (displacements landing outside the 7x7 patch multiply zero padding and
  drop out; W3 keeps the 625 surviving taps).

Per-core mapping (pure data parallelism over B, 64 samples/core), bf16:
  host: pack both patches bf16-cast and channel-pair-interleaved into
        Y[b, p, 196] with Y[b,p] = [p1[b,2p] | p2[b,2p] | p1[b,2p+1] |
        p2[b,2p+1]] -> 392B contiguous DMA descriptors per (b,p).
  stage 1: per b, two accumulating matmuls (c-even, c-odd; K=64):
        lhsT=p2-slice [64, yx=49], rhs=p1-slice [64, ij=49]
        -> PSUM [yx=49, ij=49] = A^T[b]; 4 samples per PSUM tile,
        batch-copied (f32->bf16) into SBUF acat[yx, b, ij].
  stage 2: 49 accumulating matmuls over ij: lhsT=Wst2[:, ij*4:(ij+1)*4]
        ([yx=49, n=4] bf16), rhs=acat[:, :, ij] ([yx=49, b=64] bf16)
        -> PSUM [4, 64] f32; bias added on the DVE copy out.
"""

import numpy as np

import concourse.bass as bass
import concourse.mybir as mybir
import concourse.tile as tile
from concourse import bacc
from concourse.bass_utils import run_bass_kernel_spmd

N_CORES = 8
B, C, HW = 512, 128, 49
BS = B // N_CORES   # 64 samples per core
CP = C // 2         # 64 partitions, 2 channels each
FW = 4 * HW         # 196 packed row: p1/even, p2/even, p1/odd, p2/odd
PAD = 20
GROUPS = 4          # input DMA batching: GROUPS loads of GB samples each
GB = BS // GROUPS
PSB = 4             # samples per PSUM tile / per batched copy

_F32 = mybir.dt.float32
_BF16 = mybir.dt.bfloat16


def _build_wst2(w_bbox: np.ndarray) -> np.ndarray:
    """Gather w_bbox [4, 21609] into Wst2 [yx=49, ij*4+n] (fp32)."""
    W3 = np.zeros((4, 49, 49), np.float32)
    for i in range(7):
        for j in range(7):
            for y in range(7):
                for x in range(7):
                    if (y - i) % 2 == 0 and (x - j) % 2 == 0:
                        p = (y - i + PAD) // 2
                        q = (x - j + PAD) // 2
                        W3[:, i * 7 + j, y * 7 + x] = w_bbox[
                            :, ((p * 21 + q) * 7 + i) * 7 + j
                        ]
    # Wst2[yx, ij*4 + n] = W3[n, ij, yx]
    return np.ascontiguousarray(W3.transpose(2, 1, 0).reshape(49, 196))


def build_nc() -> bass.Bass:
    nc = bacc.Bacc("TRN2", target_bir_lowering=False, debug=False)
    pp = nc.dram_tensor("pp", [BS, CP, FW], _BF16, kind="ExternalInput")
    wst2 = nc.dram_tensor("wst2", [49, 196], _BF16, kind="ExternalInput")
    bias = nc.dram_tensor("bias", [4, 1], _F32, kind="ExternalInput")
    out = nc.dram_tensor("out", [4, BS], _F32, kind="ExternalOutput")

    NCHAIN = 4  # independent stage-2 accumulation chains (hide LDW latency)
    with tile.TileContext(nc) as tc:
        with (
            tc.tile_pool(name="consts", bufs=1) as consts,
            tc.tile_pool(name="loads", bufs=GROUPS) as loads,
            tc.tile_pool(name="acat", bufs=1) as acat_pool,
        ):
            w_t = consts.tile([49, 196], _BF16)
            nc.sync.dma_start(out=w_t[:], in_=wst2[:])
            b_t = consts.tile([4, 1], _F32)
            nc.sync.dma_start(out=b_t[:], in_=bias[:])

            acat = acat_pool.tile([49, BS, HW], _BF16)  # [yx, b, ij]

            ppr = pp[:].rearrange("b p f -> p b f")
            with tc.tile_pool(
                name="ps1", bufs=2, space=bass.MemorySpace.PSUM
            ) as ps1:
                ps = None
                for g in range(GROUPS):
                    t = loads.tile([CP, GB, FW], _BF16, tag="t")
                    eng = nc.sync if g % 2 == 0 else nc.scalar
                    eng.dma_start(out=t[:], in_=ppr[:, g * GB : (g + 1) * GB, :])
                    for k in range(GB):
                        bb = g * GB + k
                        j = bb % PSB
                        if j == 0:
                            ps = ps1.tile([49, PSB, 512], _F32)
                        nc.tensor.matmul(
                            ps[:, j, 0:HW],
                            t[:, k, 49:98],     # p2, even channels
                            t[:, k, 0:49],      # p1, even channels
                            start=True,
                            stop=False,
                        )
                        nc.tensor.matmul(
                            ps[:, j, 0:HW],
                            t[:, k, 147:196],   # p2, odd channels
                            t[:, k, 98:147],    # p1, odd channels
                            start=False,
                            stop=True,
                        )
                        if j == PSB - 1:
                            ceng = nc.vector if (bb // PSB) % 2 == 0 else nc.scalar
                            src = ps[:, :, 0:HW]
                            dst = acat[:, bb - (PSB - 1) : bb + 1, :]
                            if ceng is nc.vector:
                                ceng.tensor_copy(dst, src)
                            else:
                                ceng.copy(dst, src)
            with tc.tile_pool(
                name="ps2", bufs=1, space=bass.MemorySpace.PSUM
            ) as ps2:
                chains = [
                    ps2.tile([4, BS], _F32, name=f"chain{c}", tag=f"c{c}")
                    for c in range(NCHAIN)
                ]
                for ij in range(HW):
                    c = ij % NCHAIN
                    nc.tensor.matmul(
                        chains[c][:],
                        w_t[:, ij * 4 : (ij + 1) * 4],
                        acat[:, :, ij],
                        start=(ij < NCHAIN),
                        stop=(ij + NCHAIN >= HW),
                    )
                # combine chains (only one PSUM input per DVE op) + bias
                out_sb = consts.tile([4, BS], _F32)
                nc.vector.tensor_scalar_add(out_sb[:], chains[0][:], b_t[:])
                for c in range(1, NCHAIN):
                    nc.vector.tensor_add(out_sb[:], chains[c][:], out_sb[:])
                nc.sync.dma_start(out=out[:], in_=out_sb[:])
    nc.compile()
    return nc


def _prep_inputs(inputs):
    import ml_dtypes

    p1 = np.asarray(inputs["patch1"], np.float32).reshape(B, C, HW)
    p2 = np.asarray(inputs["patch2"], np.float32).reshape(B, C, HW)
    bf = ml_dtypes.bfloat16
    Y = np.empty((B, CP, FW), bf)
    Y[:, :, 0:49] = p1[:, 0::2, :]
    Y[:, :, 49:98] = p2[:, 0::2, :]
    Y[:, :, 98:147] = p1[:, 1::2, :]
    Y[:, :, 147:196] = p2[:, 1::2, :]
    wst2 = _build_wst2(np.asarray(inputs["w_bbox"], np.float32)).astype(bf)
    bias = np.ascontiguousarray(
        np.asarray(inputs["b_bbox"], np.float32).reshape(4, 1)
    )
    in_maps = []
    for c in range(N_CORES):
        sl = slice(c * BS, (c + 1) * BS)
        in_maps.append(
            {
                "pp": np.ascontiguousarray(Y[sl]),
                "wst2": wst2,
                "bias": bias,
            }
        )
    return in_maps


def _run(inputs, trace: bool = False):
    nc = build_nc()
    in_maps = _prep_inputs(inputs)
    res = run_bass_kernel_spmd(
        nc, in_maps, core_ids=list(range(N_CORES)), trace=trace
    )
    out = np.concatenate(
        [res.results[c]["out"].T for c in range(N_CORES)], axis=0
    ).astype(np.float32)
    return out, res


def kernel(**inputs) -> np.ndarray:
    out, _ = _run(inputs, trace=False)
    return out


# revision 10
# speedup vs baseline: 1.7650x; 1.1109x over previous
"""Trainium2 Bass kernel for nn_CorrelationHead (8-core SPMD, data parallel over B).

Math reformulation (validated ~1e-6 vs the jax reference in fp32):
  corr[b,p,q,i,j] = sum_c patch1[b,c,i,j] * patch2[b,c, i+2p-20, j+2q-20]
  out[b,n] = sum w[n,:]*corr[b,:] + bias[n]
           = sum_{ij,yx} (P1[b]^T P2[b])[ij,yx] * W3[n,ij,yx] + bias[n]
  where W3 gathers w_bbox onto the 49x49 (ij,yx) grid (displacements that
  land outside the 7x7 patch hit zero padding and drop out).

Device mapping per core (64 samples), bf16, raw bass (hand-rolled sems):
  - host packs both patches channel-pair-interleaved: Y[b,p,196] =
    [p1[b,2p] | p2[b,2p] | p1[b,2p+1] | p2[b,2p+1]]  -> 392B-contiguous
    DMA descriptors; loaded as 16 sliced DMAs alternating the two HWDGE
    rings so the PE can chase the stream.
  - stage 1: per sample two accumulating K=64 matmuls (even/odd channel
    halves) -> PSUM A^T[b] [yx=49, ij=49]; 4 samples per PSUM slot-group,
    batch-cast (f32->bf16) to SBUF acat[yx,b,ij], alternating DVE/ACT.
  - stage 2: 49 accumulating matmuls contract ij (lhsT = 4 columns of the
    gathered weights, rhs = acat[:,:,ij]), interleaved over 4 independent
    accumulation chains pinned to distinct PE column strips via
    tile_position=(0,32c) so each strip's weight buffer loads while other
    strips compute; chain partials land at partitions 32c..32c+3.
  - a final selector matmul (0/1 matrix) sums the 4 chains across
    partitions; bias is added on the DVE copy out.
"""

import numpy as np

import concourse.bass as bass
import concourse.mybir as mybir
from concourse import bacc
from concourse.bass_utils import run_bass_kernel_spmd

N_CORES = 8
B, C, HW = 512, 128, 49
BS = B // N_CORES   # 64 samples per core
CP = C // 2         # 64 partitions, 2 channels each
FW = 4 * HW         # 196 packed row: p1/even, p2/even, p1/odd, p2/odd
PAD = 20
GROUPS = 4
GB = BS // GROUPS   # 16
PSB = 4             # samples per PSUM slot-group
NCHAIN = 4

_F32 = mybir.dt.float32
_BF16 = mybir.dt.bfloat16


def _build_wst2(w_bbox: np.ndarray) -> np.ndarray:
    W3 = np.zeros((4, 49, 49), np.float32)
    for i in range(7):
        for j in range(7):
            for y in range(7):
                for x in range(7):
                    if (y - i) % 2 == 0 and (x - j) % 2 == 0:
                        p = (y - i + PAD) // 2
                        q = (x - j + PAD) // 2
                        W3[:, i * 7 + j, y * 7 + x] = w_bbox[
                            :, ((p * 21 + q) * 7 + i) * 7 + j
                        ]
    return np.ascontiguousarray(W3.transpose(2, 1, 0).reshape(49, 196))


def build_nc() -> bass.Bass:
    nc = bacc.Bacc("TRN2", target_bir_lowering=False, debug=False)
    pp = nc.dram_tensor("pp", [BS, CP, FW], _BF16, kind="ExternalInput")
    wst2 = nc.dram_tensor("wst2", [49, 196], _BF16, kind="ExternalInput")
    seld = nc.dram_tensor("seld", [128, 4], _BF16, kind="ExternalInput")
    bias = nc.dram_tensor("bias", [4, 1], _F32, kind="ExternalInput")
    out = nc.dram_tensor("out", [4, BS], _F32, kind="ExternalOutput")

    ppr = pp[:].rearrange("b p f -> p b f")

    from contextlib import ExitStack

    with ExitStack() as ctx:
        ts_ = [
            ctx.enter_context(nc.sbuf_tensor(f"t{g}", [CP, GB, FW], _BF16))
            for g in range(GROUPS)
        ]
        t0, t1, t2, t3 = ts_
        acat = ctx.enter_context(nc.sbuf_tensor("acat", [49, BS, HW], _BF16))
        w_t = ctx.enter_context(nc.sbuf_tensor("w_t", [49, 196], _BF16))
        sel_w = ctx.enter_context(nc.sbuf_tensor("sel_w", [128, 4], _BF16))
        sel_sb = ctx.enter_context(nc.sbuf_tensor("sel_sb", [128, BS], _BF16))
        b_t = ctx.enter_context(nc.sbuf_tensor("b_t", [4, 1], _F32))
        out_sb = ctx.enter_context(nc.sbuf_tensor("out_sb", [4, BS], _F32))
        ps = ctx.enter_context(nc.psum_tensor("ps", [128, 8, 512], _F32))
        (sW1, sW2, sMM, sCastD, sCastA, sS2, sOut, sDone,
         sW3, sSel, sS3) = (
            ctx.enter_context(nc.semaphore(nm))
            for nm in (
                "sW1", "sW2",
                "sMM", "sCastD", "sCastA", "sS2", "sOut", "sDone",
                "sW3", "sSel", "sS3",
            )
        )
        sD = [
            ctx.enter_context(nc.semaphore(f"sD{i}")) for i in range(16)
        ]
        block = ctx.enter_context(nc.Block())

        @block.sync
        def _(sync):
            sync.dma_start(out=w_t[:], in_=wst2[:]).then_inc(sW1, 16)
            sync.dma_start(out=b_t[:], in_=bias[:]).then_inc(sW2, 16)
            sync.dma_start(out=sel_w[:], in_=seld[:]).then_inc(sW3, 16)
            for sl in range(0, 16, 2):
                sync.dma_start(
                    out=ts_[sl // 4][:, (sl % 4) * PSB : (sl % 4 + 1) * PSB, :],
                    in_=ppr[:, sl * PSB : (sl + 1) * PSB, :],
                ).then_inc(sD[sl], 16)
            sync.wait_ge(sOut, 1)
            sync.dma_start(out=out[:], in_=out_sb[:]).then_inc(sDone, 16)
            sync.wait_ge(sDone, 16)

        @block.scalar
        def _(scalar):
            for sl in range(1, 16, 2):
                scalar.dma_start(
                    out=ts_[sl // 4][:, (sl % 4) * PSB : (sl % 4 + 1) * PSB, :],
                    in_=ppr[:, sl * PSB : (sl + 1) * PSB, :],
                ).then_inc(sD[sl], 16)
            for T in range(1, 16, 2):  # odd slot-groups cast on ACT
                scalar.wait_ge(sMM, T + 1)
                nc.scalar.copy(
                    acat[:, T * PSB : (T + 1) * PSB, :],
                    ps[0:49, (T % 2) * 4 : (T % 2) * 4 + 4, 0:HW],
                ).then_inc(sCastA, 1)

        @block.tensor
        def _(tensor):
            for g in range(GROUPS):
                t = ts_[g]
                for k in range(GB):
                    bb = g * GB + k
                    T, j = bb // PSB, bb % PSB
                    if j == 0:
                        tensor.wait_ge(sD[T], 16)
                    if j == 0 and T >= 2:
                        # reuse of PSUM slot T%2: wait for cast of tile T-2
                        if T % 2 == 0:
                            tensor.wait_ge(sCastD, (T - 2) // 2 + 1)
                        else:
                            tensor.wait_ge(sCastA, (T - 2) // 2 + 1)
                    slot = T % 2
                    nc.tensor.matmul(
                        ps[0:49, slot * 4 + j, 0:HW],
                        t[:, k, 49:98],
                        t[:, k, 0:49],
                        start=True,
                        stop=False,
                    )
                    mm2 = nc.tensor.matmul(
                        ps[0:49, slot * 4 + j, 0:HW],
                        t[:, k, 147:196],
                        t[:, k, 98:147],
                        start=False,
                        stop=True,
                    )
                    if j == PSB - 1:
                        mm2.then_inc(sMM, 1)
            # stage 2: needs all of acat + w_t
            tensor.wait_ge(sCastD, 8)
            tensor.wait_ge(sCastA, 8)
            tensor.wait_ge(sW1, 16)
            last = None
            for ij in range(HW):
                c = ij % NCHAIN
                last = nc.tensor.matmul(
                    ps[32 * c : 32 * c + 4, c, 0:BS],
                    w_t[:, ij * 4 : (ij + 1) * 4],
                    acat[:, :, ij],
                    start=(ij < NCHAIN),
                    stop=(ij + NCHAIN >= HW),
                    tile_position=(0, 32 * c),
                )
            last.then_inc(sS2, 1)
            tensor.wait_ge(sSel, 1)
            tensor.wait_ge(sW3, 16)
            nc.tensor.matmul(
                ps[0:4, 7, 0:BS], sel_w[:], sel_sb[:], start=True, stop=True
            ).then_inc(sS3, 1)

        @block.vector
        def _(vector):
            nc.vector.memset(sel_sb[:], 0.0)
            for T in range(0, 16, 2):  # even slot-groups cast on DVE
                vector.wait_ge(sMM, T + 1)
                nc.vector.tensor_copy(
                    acat[:, T * PSB : (T + 1) * PSB, :],
                    ps[0:49, (T % 2) * 4 : (T % 2) * 4 + 4, 0:HW],
                ).then_inc(sCastD, 1)
            vector.wait_ge(sS2, 1)
            last_cast = None
            for c in range(NCHAIN):
                last_cast = nc.vector.tensor_copy(
                    sel_sb[32 * c : 32 * c + 4, :],
                    ps[32 * c : 32 * c + 4, c, 0:BS],
                )
            last_cast.then_inc(sSel, 1)
            vector.wait_ge(sS3, 1)
            vector.wait_ge(sW2, 16)
            nc.vector.tensor_scalar_add(
                out_sb[:], ps[0:4, 7, 0:BS], b_t[:]
            ).then_inc(sOut, 1)

    nc.compile()
    return nc


def _prep_inputs(inputs):
    import ml_dtypes

    p1 = np.asarray(inputs["patch1"], np.float32).reshape(B, C, HW)
    p2 = np.asarray(inputs["patch2"], np.float32).reshape(B, C, HW)
    bf = ml_dtypes.bfloat16
    Y = np.empty((B, CP, FW), bf)
    Y[:, :, 0:49] = p1[:, 0::2, :]
    Y[:, :, 49:98] = p2[:, 0::2, :]
    Y[:, :, 98:147] = p1[:, 1::2, :]
    Y[:, :, 147:196] = p2[:, 1::2, :]
    wst2 = _build_wst2(np.asarray(inputs["w_bbox"], np.float32)).astype(bf)
    seld = np.zeros((128, 4), bf)
    for c in range(NCHAIN):
        for n in range(4):
            seld[32 * c + n, n] = 1
    bias = np.ascontiguousarray(
        np.asarray(inputs["b_bbox"], np.float32).reshape(4, 1)
    )
    in_maps = []
    for c in range(N_CORES):
        sl = slice(c * BS, (c + 1) * BS)
        in_maps.append(
            {
                "pp": np.ascontiguousarray(Y[sl]),
                "wst2": wst2,
                "seld": seld,
                "bias": bias,
            }
        )
    return in_maps


def _run(inputs, trace: bool = False):
    nc = build_nc()
    in_maps = _prep_inputs(inputs)
    res = run_bass_kernel_spmd(
        nc, in_maps, core_ids=list(range(N_CORES)), trace=trace
    )
    out = np.concatenate(
        [res.results[c]["out"].T for c in range(N_CORES)], axis=0
    ).astype(np.float32)
    return out, res


def kernel(**inputs) -> np.ndarray:
    out, _ = _run(inputs, trace=False)
    return out
